# revision 32
# baseline (speedup 1.0000x reference)
"""Trainium2 Bass kernel for a single causal self-attention head.

Reference computation (fp32):
    Q = q @ Wq; K = q @ Wk; V = q @ Wv          # q: [B, T, D]
    scores = Q K^T / sqrt(D)  (causal masked)
    out = softmax(scores) @ V                    # [B, T, dv]

Shapes are hardcoded: B=512, T=200, D=1024, dk=dv=64, 8 NeuronCores,
batch-sharded 64 per core (pure data parallel, weights replicated).

Per-core dataflow (64 batches processed as 32 pairs, software-pipelined:
attention for pair p-1 is emitted between projection rounds so the PE
never stalls on the PSUM-evacuation / exp chain):
  - host feeds qT laid out [b, 128, 8*200] so every DMA run is a full
    contiguous 6.4KB per partition (128 descriptors per batch load)
  - projections: stationary [Wq|Wk] (and Wv) per 128-row d-tile, moving
    operand is qT for a PAIR of batches (N=400) in float32r (full-rate
    fp32 with 11-bit mantissa; host pre-rounds so results are exact)
    accumulating over 8 d-tiles -> PSUM holds [Q^T; K^T] stacked and V^T
  - scores^T = K Q^T per s-tile (E=[s,t] layout so softmax sums become a
    ones-column matmul); exp on ScalarE (no max subtraction needed:
    |scores| is O(1) by construction), causal mask via precomputed 0/1
    mask multiply
  - V^T transposed back to [s, v] on the PE, ones column appended; then
    U = E^T @ [V|1] accumulates both the numerator and the softmax
    denominator; final normalize is fused into the PSUM->SBUF copy via a
    per-partition reciprocal scale on ScalarE.
"""

import numpy as np

import concourse.bass as bass
import concourse.tile as tile
from concourse import bacc, mybir
from concourse.bass_utils import run_bass_kernel_spmd

B, T, D = 512, 200, 1024
DK = 64
N_CORES = 8
B_CORE = B // N_CORES  # 64
ND = D // 128  # 8 d-tiles
F32 = mybir.dt.float32
F32R = mybir.dt.float32r

# [(start, width)] tiles of the T=200 axis on 128 partitions (s axis)
T_TILES = [(0, 128), (128, 72)]
# Output rows are computed in two interleaved tiles (even t, odd t) so both
# pack into one SBUF tile whose per-partition 512B run is contiguous in HBM.


def build_nc(n_batch=B_CORE, use_f32r=True, repeat=1):
    """Build the per-core Bass module. Same program on all cores (SPMD)."""
    nc = bacc.Bacc("TRN2")

    # qT is host-prepped as [b, 128, ND*T]: partition p holds d-tiles
    # d*128+p, all contiguous per partition for 1-descriptor-per-partition
    # DMAs.
    qT = nc.dram_tensor("qT", [n_batch, 128, ND * T], F32, kind="ExternalInput")
    wqk = nc.dram_tensor("wqk", [D, 128], F32, kind="ExternalInput")
    wv = nc.dram_tensor("wv", [D, DK], F32, kind="ExternalInput")
    mask0 = nc.dram_tensor("mask0", [128, T], F32, kind="ExternalInput")
    mask1 = nc.dram_tensor("mask1", [72, T], F32, kind="ExternalInput")
    ident = nc.dram_tensor("ident", [128, 128], F32, kind="ExternalInput")
    shmat = nc.dram_tensor("shmat", [128, 64], F32, kind="ExternalInput")
    out = nc.dram_tensor("out", [n_batch, T, DK], F32, kind="ExternalOutput")

    assert n_batch % 2 == 0
    n_pair = n_batch // 2
    mmdt = F32R if use_f32r else F32

    with tile.TileContext(nc) as tc:
        with (
            tc.tile_pool(name="singles", bufs=1) as singles,
            tc.tile_pool(name="qt", bufs=3) as qt_pool,
            tc.tile_pool(name="sb", bufs=3) as sb_pool,
            tc.tile_pool(name="esb", bufs=4) as esb_pool,
            tc.tile_pool(name="vsb", bufs=4) as vsb_pool,
            tc.tile_pool(name="osb", bufs=12) as osb_pool,
            tc.tile_pool(name="ps_proj", bufs=1, space="PSUM") as ps_proj,
            tc.tile_pool(name="ps_vtr", bufs=1, space="PSUM") as ps_vtr,
            tc.tile_pool(name="ps_kt", bufs=1, space="PSUM") as ps_kt,
            tc.tile_pool(name="ps_e", bufs=2, space="PSUM") as ps_e,
            tc.tile_pool(name="ps_u", bufs=2, space="PSUM") as ps_u,
        ):
            # ---- constants, loaded once ----
            wqk_sb = singles.tile([128, ND, 128], mmdt)
            nc.sync.dma_start(
                out=wqk_sb, in_=wqk.rearrange("(d p) j -> p d j", p=128).bitcast(mmdt)
            )
            wv_sb = singles.tile([128, ND, DK], mmdt)
            nc.sync.dma_start(
                out=wv_sb, in_=wv.rearrange("(d p) j -> p d j", p=128).bitcast(mmdt)
            )
            m0_sb = singles.tile([128, T], F32)
            nc.sync.dma_start(out=m0_sb, in_=mask0[:, :])
            m1_sb = singles.tile([72, T], F32)
            nc.sync.dma_start(out=m1_sb, in_=mask1[:, :])
            id_sb = singles.tile([128, 128], F32)
            nc.sync.dma_start(out=id_sb, in_=ident[:, :])
            sh_sb = singles.tile([128, 64], F32R)
            nc.sync.dma_start(out=sh_sb, in_=shmat[:, :].bitcast(F32R))

            def emit_projection(p):
                """Load qT for pair p, project, evacuate. Returns tiles the
                attention stage needs."""
                # [pp, b, d*t]: per batch a fully contiguous 6.4KB/partition
                qt = qt_pool.tile([128, 2, ND * T], mmdt)
                H = ND * T // 2
                for bi in range(2):
                    for h in range(2):
                        nc.sync.dma_start(
                            out=qt[:, bi, h * H : (h + 1) * H],
                            in_=qT[2 * p + bi][:, h * H : (h + 1) * H].bitcast(mmdt),
                        )

                # qk_ps rows 0:64 = Q^T, rows 64:128 = K^T (both batches)
                qk_ps = ps_proj.tile([128, 2 * T], F32)
                vt_ps = ps_proj.tile([64, 2 * T], F32, tag="vt_ps")
                qt4 = qt.rearrange("pp b (d t) -> pp b d t", d=ND)
                for d in range(ND):
                    rhs = qt4[:, :, d, :]  # [128, 2, 200] strided
                    nc.tensor.matmul(
                        qk_ps, wqk_sb[:, d, :], rhs,
                        start=(d == 0), stop=(d == ND - 1),
                    )
                    nc.tensor.matmul(
                        vt_ps, wv_sb[:, d, :], rhs,
                        start=(d == 0), stop=(d == ND - 1),
                    )

                # evacuate PSUM; shift K^T down to partition base 0
                qk_sb = sb_pool.tile([128, 2 * T], F32R)
                nc.scalar.copy(qk_sb, qk_ps)
                vt_sb = sb_pool.tile([64, 2 * T], F32, tag="vt_sb")
                nc.vector.tensor_copy(vt_sb[:, 0:T], vt_ps[:, 0:T])
                nc.vector.tensor_copy(vt_sb[:, T : 2 * T], vt_ps[:, T : 2 * T])
                ks_ps = ps_kt.tile([64, 2 * T], F32, tag="ks_ps")
                nc.tensor.matmul(ks_ps, sh_sb, qk_sb, start=True, stop=True)
                kt_sb = sb_pool.tile([64, 2 * T], F32R, tag="kt_sb")
                nc.scalar.copy(kt_sb[:, 0:T], ks_ps[:, 0:T])
                nc.scalar.copy(kt_sb[:, T : 2 * T], ks_ps[:, T : 2 * T])
                return qk_sb, vt_sb, kt_sb

            def emit_attention(p, qk_sb, vt_sb, kt_sb):
                for bi in range(2):
                    b = 2 * p + bi
                    c0 = bi * T  # column offset of this batch in pair tiles

                    # V^T -> V (plus ones column) via PE transpose
                    vtr = ps_vtr.tile([128, 128], F32)
                    nc.tensor.transpose(
                        vtr[:, 0:64],
                        vt_sb[0:64, c0 : c0 + 128],
                        id_sb[0:64, 0:64],
                    )
                    nc.tensor.transpose(
                        vtr[0:72, 64:128],
                        vt_sb[0:64, c0 + 128 : c0 + 200],
                        id_sb[0:64, 0:64],
                    )
                    v0 = vsb_pool.tile([128, 65], F32, tag="v0")
                    nc.vector.tensor_copy(v0[:, 0:64], vtr[:, 0:64])
                    nc.vector.memset(v0[:, 64:65], 1.0)
                    v1 = vsb_pool.tile([72, 65], F32, tag="v1")
                    nc.vector.tensor_copy(v1[:, 0:64], vtr[0:72, 64:128])
                    nc.vector.memset(v1[:, 64:65], 1.0)
                    vaug = [v0, v1]

                    # scores^T (E) per s-tile: exp(K Q^T / 32) * causal mask.
                    # The matmul streams a 256-wide window of the pair-wide
                    # Q^T covering this batch's 200 columns: N>=256 keeps
                    # fp32r at 1 cycle/row with minimal wasted columns.
                    w0 = 0 if bi == 0 else 2 * T - 256
                    e_tiles = []
                    for si, (s0, sw) in enumerate(T_TILES):
                        e_ps = ps_e.tile([sw, 256], F32, tag="e_ps")
                        nc.tensor.matmul(
                            e_ps,
                            kt_sb[0:64, c0 + s0 : c0 + s0 + sw],
                            qk_sb[0:64, w0 : w0 + 256],
                            start=True,
                            stop=True,
                        )
                        e_sb = esb_pool.tile([sw, T], F32, tag="e_sb")
                        nc.scalar.activation(
                            e_sb, e_ps[:, c0 - w0 : c0 - w0 + T],
                            mybir.ActivationFunctionType.Exp,
                            scale=1.0 / 32.0,
                        )
                        msk = m0_sb if si == 0 else m1_sb
                        nc.vector.tensor_mul(e_sb, e_sb, msk)
                        e_tiles.append(e_sb)

                    # U = E^T @ [V | 1]; normalize; store.
                    # Two interleaved 100-row t-tiles (even/odd) pack into one
                    # o_sb: partition p holds rows 2p and 2p+1 -> one DMA per
                    # batch with 512B contiguous runs.
                    o_sb = osb_pool.tile([100, 2, DK], F32, tag="o_sb")
                    e_pairs = [
                        e.rearrange("s (t c) -> s t c", c=2) for e in e_tiles
                    ]
                    for ci in range(2):
                        u_ps = ps_u.tile([100, 65], F32, tag="u_ps")
                        for si, (s0, sw) in enumerate(T_TILES):
                            nc.tensor.matmul(
                                u_ps,
                                e_pairs[si][:, :, ci],
                                vaug[si],
                                start=(si == 0),
                                stop=(si == 1),
                            )
                        r_sb = osb_pool.tile([100, 1], F32, tag="r_sb")
                        nc.vector.reciprocal(r_sb, u_ps[:, 64:65])
                        nc.scalar.activation(
                            o_sb[:, ci, :], u_ps[:, 0:64],
                            mybir.ActivationFunctionType.Copy, scale=r_sb,
                        )
                    nc.gpsimd.dma_start(
                        out=out[b].rearrange("(t c) v -> t (c v)", c=2), in_=o_sb
                    )

            # software pipeline: projections run one pair ahead of attention
            for _rep in range(repeat):
                prev = None
                for p in range(n_pair):
                    tiles = emit_projection(p)
                    if prev is not None:
                        emit_attention(p - 1, *prev)
                    prev = tiles
                emit_attention(n_pair - 1, *prev)

    nc.compile()
    return nc


def round_f32r(a):
    """Round fp32 to the PE's fp32r format (11-bit mantissa, RNE)."""
    b = np.ascontiguousarray(a, dtype=np.float32).view(np.uint32)
    r = (b + 0x7FF + ((b >> 12) & 1)) & np.uint32(0xFFFFF000)
    return r.astype(np.uint32).view(np.float32)


def _host_inputs(q, Wq, Wk, Wv, use_f32r=True):
    """Shared (replicated) device inputs + per-core qT shards."""
    wqk = np.ascontiguousarray(np.concatenate([Wq, Wk], axis=1), dtype=np.float32)
    wv = np.ascontiguousarray(Wv, dtype=np.float32)
    if use_f32r:
        wqk, wv = round_f32r(wqk), round_f32r(wv)
    t_idx = np.arange(T)[None, :]
    m0 = (t_idx >= np.arange(128)[:, None]).astype(np.float32)
    m1 = (t_idx >= (128 + np.arange(72))[:, None]).astype(np.float32)
    ident = np.eye(128, dtype=np.float32)
    shmat = np.zeros((128, 64), dtype=np.float32)
    shmat[np.arange(64) + 64, np.arange(64)] = 1.0
    # [B, T, D] -> [B, D, T] -> [B, ND, 128, T] -> [B, 128, ND, T]:
    # partition p holds rows d*128+p of q^T, contiguous per partition.
    nb = q.shape[0]
    qT = np.ascontiguousarray(
        q.transpose(0, 2, 1)
        .reshape(nb, ND, 128, T)
        .transpose(0, 2, 1, 3)
        .reshape(nb, 128, ND * T)
    )
    if use_f32r:
        qT = round_f32r(qT)
    return qT, {
        "wqk": wqk, "wv": wv, "mask0": m0, "mask1": m1,
        "ident": ident, "shmat": shmat,
    }


_NC_CACHE = {}


def _get_nc(n_batch=B_CORE, use_f32r=True, repeat=1):
    key = (n_batch, use_f32r, repeat)
    if key not in _NC_CACHE:
        _NC_CACHE[key] = build_nc(n_batch, use_f32r, repeat)
    return _NC_CACHE[key]


def kernel(q, Wq, Wk, Wv):
    q = np.asarray(q, dtype=np.float32)
    qT, shared = _host_inputs(q, np.asarray(Wq), np.asarray(Wk), np.asarray(Wv))

    nc = _get_nc()
    in_maps = [
        {"qT": np.ascontiguousarray(qT[c * B_CORE : (c + 1) * B_CORE]), **shared}
        for c in range(N_CORES)
    ]
    res = run_bass_kernel_spmd(nc, in_maps, core_ids=list(range(N_CORES)))
    return np.concatenate([r["out"] for r in res.results], axis=0)


# revision 37
# speedup vs baseline: 1.0375x; 1.0375x over previous
"""Trainium2 Bass kernel for a single causal self-attention head.

Reference computation (fp32):
    Q = q @ Wq; K = q @ Wk; V = q @ Wv          # q: [B, T, D]
    scores = Q K^T / sqrt(D)  (causal masked)
    out = softmax(scores) @ V                    # [B, T, dv]

Shapes are hardcoded: B=512, T=200, D=1024, dk=dv=64, 8 NeuronCores,
batch-sharded 64 per core (pure data parallel, weights replicated).

Per-core dataflow (64 batches processed as 32 pairs, software-pipelined:
attention for pair p-1 is emitted between projection rounds so the PE
never stalls on the PSUM-evacuation / exp chain):
  - host feeds qT laid out [b, 128, 8*200] so every DMA run is a full
    contiguous 6.4KB per partition (128 descriptors per batch load)
  - projections: stationary [Wq|Wk] (and Wv) per 128-row d-tile, moving
    operand is qT for a PAIR of batches (N=400) in float32r (full-rate
    fp32 with 11-bit mantissa; host pre-rounds so results are exact)
    accumulating over 8 d-tiles -> PSUM holds [Q^T; K^T] stacked and V^T
  - scores^T = K Q^T per s-tile (E=[s,t] layout so softmax sums become a
    ones-column matmul); exp on ScalarE (no max subtraction needed:
    |scores| is O(1) by construction), causal mask via precomputed 0/1
    mask multiply
  - V^T transposed back to [s, v] on the PE, ones column appended; then
    U = E^T @ [V|1] accumulates both the numerator and the softmax
    denominator; final normalize is fused into the PSUM->SBUF copy via a
    per-partition reciprocal scale on ScalarE.
"""

import numpy as np

import concourse.bass as bass
import concourse.tile as tile
from concourse import bacc, mybir
from concourse.bass_utils import run_bass_kernel_spmd

B, T, D = 512, 200, 1024
DK = 64
N_CORES = 8
B_CORE = B // N_CORES  # 64
ND = D // 128  # 8 d-tiles
F32 = mybir.dt.float32
F32R = mybir.dt.float32r

# [(start, width)] tiles of the T=200 axis on 128 partitions (s axis)
T_TILES = [(0, 128), (128, 72)]
# Output rows are computed in two interleaved tiles (even t, odd t) so both
# pack into one SBUF tile whose per-partition 512B run is contiguous in HBM.


def build_nc(n_batch=B_CORE, use_f32r=True, repeat=1):
    """Build the per-core Bass module. Same program on all cores (SPMD)."""
    nc = bacc.Bacc("TRN2")

    # qT is host-prepped as [b, 128, ND*T]: partition p holds d-tiles
    # d*128+p, all contiguous per partition for 1-descriptor-per-partition
    # DMAs.
    qT = nc.dram_tensor("qT", [n_batch, 128, ND * T], F32, kind="ExternalInput")
    wqk = nc.dram_tensor("wqk", [D, 128], F32, kind="ExternalInput")
    wv = nc.dram_tensor("wv", [D, DK], F32, kind="ExternalInput")
    mask0 = nc.dram_tensor("mask0", [128, T], F32, kind="ExternalInput")
    mask1 = nc.dram_tensor("mask1", [72, T], F32, kind="ExternalInput")
    ident = nc.dram_tensor("ident", [128, 128], F32, kind="ExternalInput")
    shmat = nc.dram_tensor("shmat", [128, 64], F32, kind="ExternalInput")
    out = nc.dram_tensor("out", [n_batch, T, DK], F32, kind="ExternalOutput")

    assert n_batch % 2 == 0
    n_pair = n_batch // 2
    mmdt = F32R if use_f32r else F32

    with tile.TileContext(nc) as tc:
        with (
            tc.tile_pool(name="singles", bufs=1) as singles,
            tc.tile_pool(name="qt", bufs=3) as qt_pool,
            tc.tile_pool(name="sb", bufs=3) as sb_pool,
            tc.tile_pool(name="esb", bufs=4) as esb_pool,
            tc.tile_pool(name="vsb", bufs=4) as vsb_pool,
            tc.tile_pool(name="osb", bufs=12) as osb_pool,
            tc.tile_pool(name="ps_proj", bufs=1, space="PSUM") as ps_proj,
            tc.tile_pool(name="ps_vtr", bufs=1, space="PSUM") as ps_vtr,
            tc.tile_pool(name="ps_kt", bufs=1, space="PSUM") as ps_kt,
            tc.tile_pool(name="ps_e", bufs=2, space="PSUM") as ps_e,
            tc.tile_pool(name="ps_u", bufs=2, space="PSUM") as ps_u,
        ):
            # ---- constants, loaded once ----
            wqk_sb = singles.tile([128, ND, 128], mmdt)
            nc.sync.dma_start(
                out=wqk_sb, in_=wqk.rearrange("(d p) j -> p d j", p=128).bitcast(mmdt)
            )
            wv_sb = singles.tile([128, ND, DK], mmdt)
            nc.sync.dma_start(
                out=wv_sb, in_=wv.rearrange("(d p) j -> p d j", p=128).bitcast(mmdt)
            )
            m0_sb = singles.tile([128, T], F32)
            nc.sync.dma_start(out=m0_sb, in_=mask0[:, :])
            m1_sb = singles.tile([72, T], F32)
            nc.sync.dma_start(out=m1_sb, in_=mask1[:, :])
            id_sb = singles.tile([128, 128], F32)
            nc.sync.dma_start(out=id_sb, in_=ident[:, :])
            sh_sb = singles.tile([128, 64], F32R)
            nc.sync.dma_start(out=sh_sb, in_=shmat[:, :].bitcast(F32R))

            def emit_projection(p):
                """Load qT for pair p, project, evacuate. Returns tiles the
                attention stage needs."""
                # [pp, b, d*t]: per batch a fully contiguous 6.4KB/partition
                qt = qt_pool.tile([128, 2, ND * T], mmdt)
                H = ND * T // 2
                for bi in range(2):
                    for h, eng in ((0, nc.gpsimd), (1, nc.sync)):
                        eng.dma_start(
                            out=qt[:, bi, h * H : (h + 1) * H],
                            in_=qT[2 * p + bi][:, h * H : (h + 1) * H].bitcast(mmdt),
                        )

                # qk_ps rows 0:64 = Q^T, rows 64:128 = K^T (both batches)
                qk_ps = ps_proj.tile([128, 2 * T], F32)
                vt_ps = ps_proj.tile([64, 2 * T], F32, tag="vt_ps")
                qt4 = qt.rearrange("pp b (d t) -> pp b d t", d=ND)
                for d in range(ND):
                    rhs = qt4[:, :, d, :]  # [128, 2, 200] strided
                    nc.tensor.matmul(
                        qk_ps, wqk_sb[:, d, :], rhs,
                        start=(d == 0), stop=(d == ND - 1),
                    )
                    nc.tensor.matmul(
                        vt_ps, wv_sb[:, d, :], rhs,
                        start=(d == 0), stop=(d == ND - 1),
                    )

                # evacuate PSUM; shift K^T down to partition base 0
                qk_sb = sb_pool.tile([128, 2 * T], F32R)
                nc.scalar.copy(qk_sb, qk_ps)
                vt_sb = sb_pool.tile([64, 2 * T], F32, tag="vt_sb")
                nc.vector.tensor_copy(vt_sb[:, 0:T], vt_ps[:, 0:T])
                nc.vector.tensor_copy(vt_sb[:, T : 2 * T], vt_ps[:, T : 2 * T])
                ks_ps = ps_kt.tile([64, 2 * T], F32, tag="ks_ps")
                nc.tensor.matmul(ks_ps, sh_sb, qk_sb, start=True, stop=True)
                kt_sb = sb_pool.tile([64, 2 * T], F32R, tag="kt_sb")
                nc.scalar.copy(kt_sb[:, 0:T], ks_ps[:, 0:T])
                nc.scalar.copy(kt_sb[:, T : 2 * T], ks_ps[:, T : 2 * T])
                return qk_sb, vt_sb, kt_sb

            def emit_attention(p, qk_sb, vt_sb, kt_sb):
                for bi in range(2):
                    b = 2 * p + bi
                    c0 = bi * T  # column offset of this batch in pair tiles

                    # V^T -> V (plus ones column) via PE transpose
                    vtr = ps_vtr.tile([128, 128], F32)
                    nc.tensor.transpose(
                        vtr[:, 0:64],
                        vt_sb[0:64, c0 : c0 + 128],
                        id_sb[0:64, 0:64],
                    )
                    nc.tensor.transpose(
                        vtr[0:72, 64:128],
                        vt_sb[0:64, c0 + 128 : c0 + 200],
                        id_sb[0:64, 0:64],
                    )
                    v0 = vsb_pool.tile([128, 65], F32, tag="v0")
                    nc.vector.tensor_copy(v0[:, 0:64], vtr[:, 0:64])
                    nc.vector.memset(v0[:, 64:65], 1.0)
                    v1 = vsb_pool.tile([72, 65], F32, tag="v1")
                    nc.vector.tensor_copy(v1[:, 0:64], vtr[0:72, 64:128])
                    nc.vector.memset(v1[:, 64:65], 1.0)
                    vaug = [v0, v1]

                    # scores^T (E) per s-tile: exp(K Q^T / 32) * causal mask.
                    # The matmul streams a 256-wide window of the pair-wide
                    # Q^T covering this batch's 200 columns: N>=256 keeps
                    # fp32r at 1 cycle/row with minimal wasted columns.
                    w0 = 0 if bi == 0 else 2 * T - 256
                    e_tiles = []
                    for si, (s0, sw) in enumerate(T_TILES):
                        e_ps = ps_e.tile([sw, 256], F32, tag="e_ps")
                        nc.tensor.matmul(
                            e_ps,
                            kt_sb[0:64, c0 + s0 : c0 + s0 + sw],
                            qk_sb[0:64, w0 : w0 + 256],
                            start=True,
                            stop=True,
                        )
                        e_sb = esb_pool.tile([sw, T], F32, tag="e_sb")
                        nc.scalar.activation(
                            e_sb, e_ps[:, c0 - w0 : c0 - w0 + T],
                            mybir.ActivationFunctionType.Exp,
                            scale=1.0 / 32.0,
                        )
                        msk = m0_sb if si == 0 else m1_sb
                        nc.vector.tensor_mul(e_sb, e_sb, msk)
                        e_tiles.append(e_sb)

                    # U = E^T @ [V | 1]; normalize; store.
                    # Two interleaved 100-row t-tiles (even/odd) pack into one
                    # o_sb: partition p holds rows 2p and 2p+1 -> one DMA per
                    # batch with 512B contiguous runs.
                    o_sb = osb_pool.tile([100, 2, DK], F32, tag="o_sb")
                    e_pairs = [
                        e.rearrange("s (t c) -> s t c", c=2) for e in e_tiles
                    ]
                    for ci in range(2):
                        u_ps = ps_u.tile([100, 65], F32, tag="u_ps")
                        for si, (s0, sw) in enumerate(T_TILES):
                            nc.tensor.matmul(
                                u_ps,
                                e_pairs[si][:, :, ci],
                                vaug[si],
                                start=(si == 0),
                                stop=(si == 1),
                            )
                        r_sb = osb_pool.tile([100, 1], F32, tag="r_sb")
                        nc.vector.reciprocal(r_sb, u_ps[:, 64:65])
                        nc.scalar.activation(
                            o_sb[:, ci, :], u_ps[:, 0:64],
                            mybir.ActivationFunctionType.Copy, scale=r_sb,
                        )
                    nc.sync.dma_start(
                        out=out[b].rearrange("(t c) v -> t (c v)", c=2), in_=o_sb
                    )

            # software pipeline: projections run one pair ahead of attention
            for _rep in range(repeat):
                prev = None
                for p in range(n_pair):
                    tiles = emit_projection(p)
                    if prev is not None:
                        emit_attention(p - 1, *prev)
                    prev = tiles
                emit_attention(n_pair - 1, *prev)

    nc.compile()
    return nc


def round_f32r(a):
    """Round fp32 to the PE's fp32r format (11-bit mantissa, RNE)."""
    b = np.ascontiguousarray(a, dtype=np.float32).view(np.uint32)
    r = (b + 0x7FF + ((b >> 12) & 1)) & np.uint32(0xFFFFF000)
    return r.astype(np.uint32).view(np.float32)


def _host_inputs(q, Wq, Wk, Wv, use_f32r=True):
    """Shared (replicated) device inputs + per-core qT shards."""
    wqk = np.ascontiguousarray(np.concatenate([Wq, Wk], axis=1), dtype=np.float32)
    wv = np.ascontiguousarray(Wv, dtype=np.float32)
    if use_f32r:
        wqk, wv = round_f32r(wqk), round_f32r(wv)
    t_idx = np.arange(T)[None, :]
    m0 = (t_idx >= np.arange(128)[:, None]).astype(np.float32)
    m1 = (t_idx >= (128 + np.arange(72))[:, None]).astype(np.float32)
    ident = np.eye(128, dtype=np.float32)
    shmat = np.zeros((128, 64), dtype=np.float32)
    shmat[np.arange(64) + 64, np.arange(64)] = 1.0
    # [B, T, D] -> [B, D, T] -> [B, ND, 128, T] -> [B, 128, ND, T]:
    # partition p holds rows d*128+p of q^T, contiguous per partition.
    nb = q.shape[0]
    qT = np.ascontiguousarray(
        q.transpose(0, 2, 1)
        .reshape(nb, ND, 128, T)
        .transpose(0, 2, 1, 3)
        .reshape(nb, 128, ND * T)
    )
    if use_f32r:
        qT = round_f32r(qT)
    return qT, {
        "wqk": wqk, "wv": wv, "mask0": m0, "mask1": m1,
        "ident": ident, "shmat": shmat,
    }


_NC_CACHE = {}


def _get_nc(n_batch=B_CORE, use_f32r=True, repeat=1):
    key = (n_batch, use_f32r, repeat)
    if key not in _NC_CACHE:
        _NC_CACHE[key] = build_nc(n_batch, use_f32r, repeat)
    return _NC_CACHE[key]


def kernel(q, Wq, Wk, Wv):
    q = np.asarray(q, dtype=np.float32)
    qT, shared = _host_inputs(q, np.asarray(Wq), np.asarray(Wk), np.asarray(Wv))

    nc = _get_nc()
    in_maps = [
        {"qT": np.ascontiguousarray(qT[c * B_CORE : (c + 1) * B_CORE]), **shared}
        for c in range(N_CORES)
    ]
    res = run_bass_kernel_spmd(nc, in_maps, core_ids=list(range(N_CORES)))
    return np.concatenate([r["out"] for r in res.results], axis=0)
